# revision 3
# baseline (speedup 1.0000x reference)
"""Multi-head attention forward on 8 TRN2 NeuronCores, data-parallel over batch.

Reference computation (per batch element b):
    qkv  = x @ qkv_w.T + qkv_b                     # [N, 3D]
    q, k = LN_headdim(q), LN_headdim(k)            # layernorm over head_dim=64
    S    = q @ k.T * hd^-0.5 ; A = softmax_j(S)    # per head
    out  = (A @ v) @ proj_w.T + proj_b             # [N, D]

Kernel strategy (one batch element per core, no collectives):
  - bf16 matmuls on TensorE; f32 statistics/softmax denominators.
  - Scores computed TRANSPOSED: ST[j,i] = k_j . q_i so that E = exp(ST*scale)
    lands in SBUF with the contraction axis j on partitions -- E is directly
    the lhsT of the attn@v matmul (no attention-matrix transpose needed).
  - Softmax denominators come free: V gets a ones-column appended, so
    psum[i, 64] = sum_j E[j,i]; normalize with a per-partition scalar.
  - No max-subtraction in softmax: q,k are layernormed so |q.k|*scale <= 8,
    exp() is safely bounded (<= e^8) in f32/bf16.
  - All operand transposes (x, weights, q/k heads, attn output) go through
    DMA-engine 128x128 bf16 transposes -- zero TensorE cycles.
"""

import sys

import numpy as np

sys.path.insert(0, "/opt/trn_rl_repo")

from contextlib import ExitStack

import concourse.bass as bass
import concourse.tile as tile
from concourse import bacc, mybir
from concourse.bass_utils import run_bass_kernel_spmd

B, N, D = 8, 1024, 768
H, HD = 12, 64
O3 = 3 * D  # 2304
P = 128
NT = N // P  # 8 token tiles
DC = D // P  # 6 contraction subtiles
EPS = 1e-5
SCALE = HD ** -0.5  # 0.125
F32 = mybir.dt.float32
BF16 = mybir.dt.bfloat16

# qkv output chunks: [start, size]; q = o[0:768), k = [768:1536), v = [1536:2304)
QKV_CHUNKS = [(0, 512), (512, 512), (1024, 512), (1536, 512), (2048, 256)]


def _bcast_ap(ap_1d, parts):
    """View a 1-D DRAM AP as [parts, n] with partition stride 0 (broadcast)."""
    return bass.AP(
        tensor=ap_1d.tensor,
        offset=ap_1d.offset,
        ap=[[0, parts]] + list(ap_1d.ap),
    )


def _build_graph():
    nc = bacc.Bacc("TRN2", target_bir_lowering=False, debug=False, num_devices=B)

    x_d = nc.dram_tensor("x", [N, D], F32, kind="ExternalInput").ap()
    qkvw_d = nc.dram_tensor("qkv_w", [O3, D], F32, kind="ExternalInput").ap()
    qkvb_d = nc.dram_tensor("qkv_b", [O3], F32, kind="ExternalInput").ap()
    projw_d = nc.dram_tensor("proj_w", [D, D], F32, kind="ExternalInput").ap()
    projb_d = nc.dram_tensor("proj_b", [D], F32, kind="ExternalInput").ap()
    gamma_d = nc.dram_tensor("qn_gamma", [HD], F32, kind="ExternalInput").ap()
    beta_d = nc.dram_tensor("qn_beta", [HD], F32, kind="ExternalInput").ap()
    out_d = nc.dram_tensor("out", [N, D], F32, kind="ExternalOutput").ap()

    with tile.TileContext(nc) as tc:
        _emit(tc, out_d, x_d, qkvw_d, qkvb_d, projw_d, projb_d, gamma_d, beta_d)

    nc.compile()
    return nc


def _emit(tc, out_d, x_d, qkvw_d, qkvb_d, projw_d, projb_d, gamma_d, beta_d):
    nc = tc.nc
    ctx = ExitStack()
    with ctx:
        const = ctx.enter_context(tc.tile_pool(name="const", bufs=1))
        wpool = ctx.enter_context(tc.tile_pool(name="wts", bufs=1))
        lpool = ctx.enter_context(tc.tile_pool(name="loads", bufs=3))
        data = ctx.enter_context(tc.tile_pool(name="data", bufs=1))
        epool = ctx.enter_context(tc.tile_pool(name="escore", bufs=2))
        qkpool = ctx.enter_context(tc.tile_pool(name="qk", bufs=2))
        tmpp = ctx.enter_context(tc.tile_pool(name="tmp", bufs=3))
        stat = ctx.enter_context(tc.tile_pool(name="stat", bufs=4))
        outp = ctx.enter_context(tc.tile_pool(name="outp", bufs=3))
        ps_mm = ctx.enter_context(tc.tile_pool(name="ps_mm", bufs=2, space="PSUM"))
        ps_st = ctx.enter_context(tc.tile_pool(name="ps_st", bufs=2, space="PSUM"))
        ps_av = ctx.enter_context(tc.tile_pool(name="ps_av", bufs=4, space="PSUM"))

        # ---- constants ----
        qkvb_bc = const.tile([P, O3], F32)
        nc.sync.dma_start(qkvb_bc[:], _bcast_ap(qkvb_d, P))
        projb_bc = const.tile([P, D], F32)
        nc.sync.dma_start(projb_bc[:], _bcast_ap(projb_d, P))
        gamma_bc = const.tile([P, HD], F32)
        nc.sync.dma_start(gamma_bc[:], _bcast_ap(gamma_d, P))
        beta_bc = const.tile([P, HD], F32)
        nc.sync.dma_start(beta_bc[:], _bcast_ap(beta_d, P))
        eps_t = const.tile([P, 1], F32)
        nc.vector.memset(eps_t[:], EPS)

        # ---- load + cast + DMA-transpose x and weights into [k, ., m] layouts ----
        xT = wpool.tile([P, DC, N], BF16)      # [d_in, d_out, t]
        qkvwT = wpool.tile([P, DC, O3], BF16)  # [d_in, d_out, o]
        projwT = wpool.tile([P, DC, D], BF16)  # [o_in, o_out, e]

        def load_cast_transpose(src_d, n_rows_tiles, dstT, dst_cols_per_tile=P):
            for rt in range(n_rows_tiles):
                t_f = lpool.tile([P, D], F32, tag="ld_f32")
                nc.sync.dma_start(t_f[:], src_d[rt * P:(rt + 1) * P, :])
                t_b = lpool.tile([P, D], BF16, tag="ld_bf16")
                nc.any.tensor_copy(t_b[:], t_f[:])
                for dc in range(DC):
                    nc.sync.dma_start(
                        dstT[:, dc, rt * P:(rt + 1) * P],
                        t_b[:, dc * P:(dc + 1) * P],
                        transpose=True,
                    )

        load_cast_transpose(x_d, NT, xT)
        load_cast_transpose(qkvw_d, O3 // P, qkvwT)
        load_cast_transpose(projw_d, D // P, projwT)

        # ---- QKV projection + bias + head-dim layernorm on q,k ----
        qn = data.tile([P, NT, D], BF16)            # [t_in, t_out, o]  (q heads)
        kn = data.tile([P, NT, D], BF16)
        vext = data.tile([P, NT, H, HD + 1], BF16)  # v with ones column
        nc.vector.memset(vext[:, :, :, HD:HD + 1], 1.0)

        for tt in range(NT):
            for (c0, cs) in QKV_CHUNKS:
                psum_full = ps_mm.tile([P, 512], F32, tag="mm", name="psum_mm")
                psum = psum_full[:, :cs]
                for dc in range(DC):
                    nc.tensor.matmul(
                        psum,
                        lhsT=xT[:, dc, tt * P:(tt + 1) * P],
                        rhs=qkvwT[:, dc, c0:c0 + cs],
                        start=(dc == 0),
                        stop=(dc == DC - 1),
                    )
                if c0 < 2 * D:
                    # q/k chunk: bias add then LN over 64-wide segments
                    nsg = cs // HD
                    tmp_c_full = tmpp.tile([P, 512], F32, tag="tmpc", name="tmp_c")
                    tmp_c = tmp_c_full[:, :cs]
                    nc.vector.tensor_add(tmp_c, psum, qkvb_bc[:, c0:c0 + cs])
                    t3 = tmp_c.rearrange("p (s h) -> p s h", h=HD)
                    sums_full = stat.tile([P, 8], F32, tag="sums", name="sums")
                    sums = sums_full[:, :nsg]
                    nc.vector.tensor_reduce(
                        sums, t3, axis=mybir.AxisListType.X, op=mybir.AluOpType.add
                    )
                    sq_full = tmpp.tile([P, 512], F32, tag="sq", name="sq")
                    sq = sq_full[:, :cs]
                    nc.scalar.square(sq, tmp_c)
                    sqs_full = stat.tile([P, 8], F32, tag="sqs", name="sqs")
                    sqs = sqs_full[:, :nsg]
                    nc.vector.tensor_reduce(
                        sqs,
                        sq.rearrange("p (s h) -> p s h", h=HD),
                        axis=mybir.AxisListType.X,
                        op=mybir.AluOpType.add,
                    )
                    mean_full = stat.tile([P, 8], F32, tag="mean", name="mean")
                    mean = mean_full[:, :nsg]
                    nc.vector.tensor_scalar_mul(mean, sums, 1.0 / HD)
                    msq_full = stat.tile([P, 8], F32, tag="msq", name="msq")
                    msq = msq_full[:, :nsg]
                    nc.vector.tensor_mul(msq, mean, mean)
                    var_full = stat.tile([P, 8], F32, tag="var", name="var")
                    var = var_full[:, :nsg]
                    nc.vector.tensor_scalar_mul(var, sqs, 1.0 / HD)
                    nc.vector.tensor_sub(var, var, msq)
                    std_full = stat.tile([P, 8], F32, tag="std", name="std")
                    std = std_full[:, :nsg]
                    nc.scalar.activation(
                        std, var, mybir.ActivationFunctionType.Sqrt, bias=eps_t[:]
                    )
                    rstd_full = stat.tile([P, 8], F32, tag="rstd", name="rstd")
                    rstd = rstd_full[:, :nsg]
                    nc.vector.reciprocal(rstd, std)
                    # normalize in place: (tmp - mean) * rstd
                    mean_b = mean[:, :, None].to_broadcast((P, nsg, HD))
                    rstd_b = rstd[:, :, None].to_broadcast((P, nsg, HD))
                    nc.vector.tensor_tensor(t3, t3, mean_b, op=mybir.AluOpType.subtract)
                    nc.vector.tensor_tensor(t3, t3, rstd_b, op=mybir.AluOpType.mult)
                    # gamma * xn + beta, cast to bf16, split at q/k boundary (o=768)
                    gamma_b = gamma_bc[:, None, :].to_broadcast((P, nsg, HD))
                    nc.gpsimd.tensor_tensor(t3, t3, gamma_b, op=mybir.AluOpType.mult)
                    spans = []
                    if c0 < D:
                        q_hi = min(c0 + cs, D)
                        spans.append((qn, c0, q_hi - c0, 0))
                    if c0 + cs > D:
                        k_lo = max(c0, D)
                        spans.append((kn, k_lo - D, c0 + cs - k_lo, k_lo - c0))
                    for (dst, d0, dlen, src_off) in spans:
                        beta_b = beta_bc[:, None, :].to_broadcast((P, dlen // HD, HD))
                        src = t3[:, src_off // HD:(src_off + dlen) // HD, :]
                        dgt = dst[:, tt, d0:d0 + dlen].rearrange(
                            "p (s h) -> p s h", h=HD
                        )
                        nc.gpsimd.tensor_tensor(dgt, src, beta_b, op=mybir.AluOpType.add)
                else:
                    # v chunk: bias add, cast bf16, scatter into 65-strided vext
                    hs = (c0 - 2 * D) // HD
                    nh = cs // HD
                    nc.vector.tensor_tensor(
                        vext[:, tt, hs:hs + nh, 0:HD],
                        psum.rearrange("p (s h) -> p s h", h=HD),
                        qkvb_bc[:, c0:c0 + cs].rearrange("p (s h) -> p s h", h=HD),
                        op=mybir.AluOpType.add,
                    )

        # ---- per-head attention (processed in head pairs: DMA transpose needs
        # 128-wide free dim, so transpose two adjacent heads' [128,128] slab) ----
        attnout = data.tile([P, NT, D], BF16)  # [t_in, t_out, o]
        for hp in range(H // 2):
            # qqT/kkT: [hd, t] per head pair; head 2hp in partitions 0:64,
            # head 2hp+1 in partitions 64:128
            qqT = qkpool.tile([P, N], BF16, tag="qqT")
            kkT = qkpool.tile([P, N], BF16, tag="kkT")
            for tt in range(NT):
                nc.sync.dma_start(
                    qqT[:, tt * P:(tt + 1) * P],
                    qn[:, tt, hp * 2 * HD:(hp + 1) * 2 * HD],
                    transpose=True,
                )
                nc.sync.dma_start(
                    kkT[:, tt * P:(tt + 1) * P],
                    kn[:, tt, hp * 2 * HD:(hp + 1) * 2 * HD],
                    transpose=True,
                )
            for hh in range(2):
                h = hp * 2 + hh
                qT = qqT[hh * HD:(hh + 1) * HD, :]
                kT = kkT[hh * HD:(hh + 1) * HD, :]
                # E[j, i] = exp(scale * k_j . q_i)
                E = epool.tile([P, NT, N], BF16, tag="E")
                for jt in range(NT):
                    for ic in range(2):
                        ps = ps_st.tile([P, 512], F32, tag="st")
                        nc.tensor.matmul(
                            ps,
                            lhsT=kT[:, jt * P:(jt + 1) * P],
                            rhs=qT[:, ic * 512:(ic + 1) * 512],
                            start=True,
                            stop=True,
                        )
                        nc.scalar.activation(
                            E[:, jt, ic * 512:(ic + 1) * 512],
                            ps,
                            mybir.ActivationFunctionType.Exp,
                            scale=SCALE,
                        )
                # out[i, :] = (sum_j E[j,i] * [v_j | 1]); normalize by col 64
                for it in range(NT):
                    pa = ps_av.tile([P, HD + 1], F32, tag="av")
                    for jt in range(NT):
                        nc.tensor.matmul(
                            pa,
                            lhsT=E[:, jt, it * P:(it + 1) * P],
                            rhs=vext[:, jt, h, :],
                            start=(jt == 0),
                            stop=(jt == NT - 1),
                        )
                    rcp = stat.tile([P, 1], F32, tag="rcp")
                    nc.vector.reciprocal(rcp, pa[:, HD:HD + 1])
                    nc.vector.tensor_scalar_mul(
                        attnout[:, it, h * HD:(h + 1) * HD], pa[:, 0:HD], scalar1=rcp
                    )

        # ---- output projection ----
        attnoutT = data.tile([P, DC, N], BF16)  # [o_in, o_out, t]
        for tt in range(NT):
            for oc in range(DC):
                nc.sync.dma_start(
                    attnoutT[:, oc, tt * P:(tt + 1) * P],
                    attnout[:, tt, oc * P:(oc + 1) * P],
                    transpose=True,
                )
        EC = 384
        for tt in range(NT):
            for ec in range(D // EC):
                ps_full = ps_mm.tile([P, 512], F32, tag="mm", name="ps_proj")
                ps = ps_full[:, :EC]
                for oc in range(DC):
                    nc.tensor.matmul(
                        ps,
                        lhsT=attnoutT[:, oc, tt * P:(tt + 1) * P],
                        rhs=projwT[:, oc, ec * EC:(ec + 1) * EC],
                        start=(oc == 0),
                        stop=(oc == DC - 1),
                    )
                ot = outp.tile([P, EC], F32, tag="outt")
                nc.vector.tensor_add(ot[:], ps, projb_bc[:, ec * EC:(ec + 1) * EC])
                nc.sync.dma_start(out_d[tt * P:(tt + 1) * P, ec * EC:(ec + 1) * EC], ot[:])


_NC_CACHE = None


def _get_nc():
    global _NC_CACHE
    if _NC_CACHE is None:
        _NC_CACHE = _build_graph()
    return _NC_CACHE


def kernel(x, qkv_w, qkv_b, proj_w, proj_b, qn_gamma, qn_beta):
    nc = _get_nc()
    shared = {
        "qkv_w": np.ascontiguousarray(qkv_w, np.float32),
        "qkv_b": np.ascontiguousarray(qkv_b, np.float32),
        "proj_w": np.ascontiguousarray(proj_w, np.float32),
        "proj_b": np.ascontiguousarray(proj_b, np.float32),
        "qn_gamma": np.ascontiguousarray(qn_gamma, np.float32),
        "qn_beta": np.ascontiguousarray(qn_beta, np.float32),
    }
    in_maps = [
        {**shared, "x": np.ascontiguousarray(x[i], np.float32)} for i in range(B)
    ]
    res = run_bass_kernel_spmd(nc, in_maps, core_ids=list(range(B)))
    return np.stack([res.results[i]["out"] for i in range(B)], axis=0)


# revision 7
# speedup vs baseline: 1.9746x; 1.9746x over previous
"""Multi-head attention forward on 8 TRN2 NeuronCores, data-parallel over batch.

Reference computation (per batch element b):
    qkv  = x @ qkv_w.T + qkv_b                     # [N, 3D]
    q, k = LN_headdim(q), LN_headdim(k)            # layernorm over head_dim=64
    S    = q @ k.T * hd^-0.5 ; A = softmax_j(S)    # per head
    out  = (A @ v) @ proj_w.T + proj_b             # [N, D]

Kernel strategy (one batch element per core, no collectives):
  - bf16 matmuls on TensorE; f32 statistics/softmax denominators.
  - Scores computed TRANSPOSED: ST[j,i] = k_j . q_i so that E = exp(ST*scale)
    lands in SBUF with the contraction axis j on partitions -- E is directly
    the lhsT of the attn@v matmul (no attention-matrix transpose needed).
  - Softmax denominators come free: V gets a ones-column appended, so
    psum[i, 64] = sum_j E[j,i]; normalize with a per-partition scalar.
  - No max-subtraction in softmax: q,k are layernormed so |q.k|*scale <= 8,
    exp() is safely bounded (<= e^8) in f32/bf16.
  - All transposes on TensorE (identity matmul), batched 4 tiles into one
    [128,512] PSUM bank with a single evacuation copy.  DMA transposes are
    avoided entirely: they shatter into 256B packets (measured 780us of DMA
    engine time for this problem).
  - Engine balance: exp on ScalarE; reductions/psum-reads on VectorE;
    SBUF-only elementwise (casts, squares, LN scale) on GpSimd.
"""

import sys

import numpy as np

sys.path.insert(0, "/opt/trn_rl_repo")

from contextlib import ExitStack

import concourse.bass as bass
import concourse.tile as tile
from concourse import bacc, mybir
from concourse.bass_utils import run_bass_kernel_spmd
from concourse.masks import make_identity

B, N, D = 8, 1024, 768
H, HD = 12, 64
O3 = 3 * D  # 2304
P = 128
NT = N // P  # 8 token tiles
DC = D // P  # 6 contraction subtiles
EPS = 1e-5
SCALE = HD ** -0.5  # 0.125
F32 = mybir.dt.float32
BF16 = mybir.dt.bfloat16

# qkv output chunks: [start, size]; q = o[0:768), k = [768:1536), v = [1536:2304)
QKV_CHUNKS = [(0, 512), (512, 512), (1024, 512), (1536, 512), (2048, 256)]


def _bcast_ap(ap_1d, parts):
    """View a 1-D DRAM AP as [parts, n] with partition stride 0 (broadcast)."""
    return bass.AP(
        tensor=ap_1d.tensor,
        offset=ap_1d.offset,
        ap=[[0, parts]] + list(ap_1d.ap),
    )


def _groups_of(n, g):
    """Split range(n) into [(start, len)] groups of at most g."""
    return [(s, min(g, n - s)) for s in range(0, n, g)]


def _build_graph(apply_gn):
    nc = bacc.Bacc("TRN2", target_bir_lowering=False, debug=False, num_devices=B)

    x_d = nc.dram_tensor("x", [N, D], F32, kind="ExternalInput").ap()
    qkvw_d = nc.dram_tensor("qkv_w", [O3, D], F32, kind="ExternalInput").ap()
    qkvb_d = nc.dram_tensor("qkv_b", [O3], F32, kind="ExternalInput").ap()
    projw_d = nc.dram_tensor("proj_w", [D, D], F32, kind="ExternalInput").ap()
    projb_d = nc.dram_tensor("proj_b", [D], F32, kind="ExternalInput").ap()
    gamma_d = nc.dram_tensor("qn_gamma", [HD], F32, kind="ExternalInput").ap()
    beta_d = nc.dram_tensor("qn_beta", [HD], F32, kind="ExternalInput").ap()
    out_d = nc.dram_tensor("out", [N, D], F32, kind="ExternalOutput").ap()

    with tile.TileContext(nc) as tc:
        _emit(tc, out_d, x_d, qkvw_d, qkvb_d, projw_d, projb_d, gamma_d, beta_d,
              apply_gn)

    nc.compile()
    return nc


def _emit(tc, out_d, x_d, qkvw_d, qkvb_d, projw_d, projb_d, gamma_d, beta_d,
          apply_gn):
    nc = tc.nc
    ctx = ExitStack()
    with ctx:
        const = ctx.enter_context(tc.tile_pool(name="const", bufs=1))
        wpool = ctx.enter_context(tc.tile_pool(name="wts", bufs=1))
        data = ctx.enter_context(tc.tile_pool(name="data", bufs=1))
        epool = ctx.enter_context(tc.tile_pool(name="escore", bufs=2))
        qkpool = ctx.enter_context(tc.tile_pool(name="qk", bufs=2))
        tmpp = ctx.enter_context(tc.tile_pool(name="tmp", bufs=3))
        stat = ctx.enter_context(tc.tile_pool(name="stat", bufs=4))
        outp = ctx.enter_context(tc.tile_pool(name="outp", bufs=3))
        ps_tr = ctx.enter_context(tc.tile_pool(name="ps_tr", bufs=2, space="PSUM"))
        ps_mm = ctx.enter_context(tc.tile_pool(name="ps_mm", bufs=2, space="PSUM"))
        ps_st = ctx.enter_context(tc.tile_pool(name="ps_st", bufs=2, space="PSUM"))
        ps_av = ctx.enter_context(tc.tile_pool(name="ps_av", bufs=2, space="PSUM"))

        # ---- constants ----
        qkvb_bc = const.tile([P, O3], F32)
        nc.sync.dma_start(qkvb_bc[:], _bcast_ap(qkvb_d, P))
        projb_bc = const.tile([P, D], F32)
        nc.sync.dma_start(projb_bc[:], _bcast_ap(projb_d, P))
        eps_t = const.tile([P, 1], F32)
        nc.vector.memset(eps_t[:], EPS)
        ident = const.tile([P, P], BF16)
        make_identity(nc, ident[:])
        if apply_gn:
            gamma_bc = const.tile([P, HD], F32)
            nc.sync.dma_start(gamma_bc[:], _bcast_ap(gamma_d, P))
            beta_bc = const.tile([P, HD], F32)
            nc.sync.dma_start(beta_bc[:], _bcast_ap(beta_d, P))

        def pe_transpose_batch(src_tiles, dst, dst_col0, evac_engine):
            """PE-transpose up to 4 [128,128] bf16 tiles through one PSUM bank;
            dst gets columns [dst_col0, dst_col0 + 128*len)."""
            ng = len(src_tiles)
            ps_full = ps_tr.tile([P, 512], BF16, tag="tr", name="ps_tr_t")
            ps = ps_full[:, :ng * P]
            for i, src in enumerate(src_tiles):
                nc.tensor.transpose(ps_full[:, i * P:(i + 1) * P], src, ident[:])
            if evac_engine is nc.scalar:
                evac_engine.copy(dst[:, dst_col0:dst_col0 + ng * P], ps)
            else:
                evac_engine.tensor_copy(dst[:, dst_col0:dst_col0 + ng * P], ps)

        # ---- load + cast + PE-transpose x and weights into [k, ., m] layouts ----
        xT = wpool.tile([P, DC, N], BF16)      # [d_in, d_out, t]
        qkvwT = wpool.tile([P, DC, O3], BF16)  # [d_in, d_out, o]
        projwT = wpool.tile([P, DC, D], BF16)  # [o_in, o_out, e]

        with tc.tile_pool(name="prep", bufs=2) as prep:
            def load_cast_transpose(src_d, n_rt, dstT):
                for (g0, gn) in _groups_of(n_rt, 4):
                    stage = prep.tile([P, 4, D], BF16, tag="stage", name="stage")
                    for i in range(gn):
                        t_f = prep.tile([P, D], F32, tag="ld_f32", name="t_f")
                        nc.sync.dma_start(
                            t_f[:], src_d[(g0 + i) * P:(g0 + i + 1) * P, :]
                        )
                        nc.gpsimd.tensor_copy(stage[:, i, :], t_f[:])
                    for dc in range(DC):
                        srcs = [stage[:, i, dc * P:(dc + 1) * P]
                                for i in range(gn)]
                        pe_transpose_batch(srcs, dstT[:, dc, :], g0 * P, nc.scalar)

            load_cast_transpose(x_d, NT, xT)
            load_cast_transpose(qkvw_d, O3 // P, qkvwT)
            load_cast_transpose(projw_d, D // P, projwT)

        # ---- QKV projection + bias + head-dim layernorm on q,k ----
        qn = data.tile([P, NT, D], BF16)            # [t_in, t_out, o]  (q heads)
        kn = data.tile([P, NT, D], BF16)
        vext = data.tile([P, NT, H, HD + 1], BF16)  # v with ones column
        nc.vector.memset(vext[:, :, :, HD:HD + 1], 1.0)

        for tt in range(NT):
            for (c0, cs) in QKV_CHUNKS:
                psum_full = ps_mm.tile([P, 512], F32, tag="mm", name="psum_mm")
                psum = psum_full[:, :cs]
                for dc in range(DC):
                    nc.tensor.matmul(
                        psum,
                        lhsT=xT[:, dc, tt * P:(tt + 1) * P],
                        rhs=qkvwT[:, dc, c0:c0 + cs],
                        start=(dc == 0),
                        stop=(dc == DC - 1),
                    )
                if c0 < 2 * D:
                    # q/k chunk: bias add then LN over 64-wide segments
                    nsg = cs // HD
                    tmp_c_full = tmpp.tile([P, 512], F32, tag="tmpc", name="tmp_c")
                    tmp_c = tmp_c_full[:, :cs]
                    nc.vector.tensor_add(tmp_c, psum, qkvb_bc[:, c0:c0 + cs])
                    t3 = tmp_c.rearrange("p (s h) -> p s h", h=HD)
                    sums_full = stat.tile([P, 8], F32, tag="sums", name="sums")
                    sums = sums_full[:, :nsg]
                    nc.vector.tensor_reduce(
                        sums, t3, axis=mybir.AxisListType.X, op=mybir.AluOpType.add
                    )
                    sq_full = tmpp.tile([P, 512], F32, tag="sq", name="sq")
                    sq = sq_full[:, :cs]
                    nc.gpsimd.tensor_mul(sq, tmp_c, tmp_c)
                    sqs_full = stat.tile([P, 8], F32, tag="sqs", name="sqs")
                    sqs = sqs_full[:, :nsg]
                    nc.vector.tensor_reduce(
                        sqs,
                        sq.rearrange("p (s h) -> p s h", h=HD),
                        axis=mybir.AxisListType.X,
                        op=mybir.AluOpType.add,
                    )
                    mean_full = stat.tile([P, 8], F32, tag="mean", name="mean")
                    mean = mean_full[:, :nsg]
                    nc.vector.tensor_scalar_mul(mean, sums, 1.0 / HD)
                    msq_full = stat.tile([P, 8], F32, tag="msq", name="msq")
                    msq = msq_full[:, :nsg]
                    nc.vector.tensor_mul(msq, mean, mean)
                    var_full = stat.tile([P, 8], F32, tag="var", name="var")
                    var = var_full[:, :nsg]
                    nc.vector.tensor_scalar_mul(var, sqs, 1.0 / HD)
                    nc.vector.tensor_sub(var, var, msq)
                    std_full = stat.tile([P, 8], F32, tag="std", name="std")
                    std = std_full[:, :nsg]
                    nc.scalar.activation(
                        std, var, mybir.ActivationFunctionType.Sqrt, bias=eps_t[:]
                    )
                    rstd_full = stat.tile([P, 8], F32, tag="rstd", name="rstd")
                    rstd = rstd_full[:, :nsg]
                    nc.vector.reciprocal(rstd, std)
                    # normalize: (tmp - mean) * rstd  (broadcast stats over HD)
                    mean_b = mean[:, :, None].to_broadcast((P, nsg, HD))
                    rstd_b = rstd[:, :, None].to_broadcast((P, nsg, HD))
                    nc.gpsimd.tensor_tensor(t3, t3, mean_b, op=mybir.AluOpType.subtract)
                    if apply_gn:
                        nc.gpsimd.tensor_tensor(t3, t3, rstd_b, op=mybir.AluOpType.mult)
                        gamma_b = gamma_bc[:, None, :].to_broadcast((P, nsg, HD))
                        nc.gpsimd.tensor_tensor(t3, t3, gamma_b, op=mybir.AluOpType.mult)
                    # write bf16 into qn/kn, splitting at the q/k boundary (o=768)
                    spans = []
                    if c0 < D:
                        q_hi = min(c0 + cs, D)
                        spans.append((qn, c0, q_hi - c0, 0))
                    if c0 + cs > D:
                        k_lo = max(c0, D)
                        spans.append((kn, k_lo - D, c0 + cs - k_lo, k_lo - c0))
                    for (dst, d0, dlen, src_off) in spans:
                        nsg_s = dlen // HD
                        src = t3[:, src_off // HD:(src_off + dlen) // HD, :]
                        dgt = dst[:, tt, d0:d0 + dlen].rearrange(
                            "p (s h) -> p s h", h=HD
                        )
                        if apply_gn:
                            beta_b = beta_bc[:, None, :].to_broadcast((P, nsg_s, HD))
                            nc.gpsimd.tensor_tensor(
                                dgt, src, beta_b, op=mybir.AluOpType.add
                            )
                        else:
                            rstd_s = rstd_b[:, src_off // HD:(src_off + dlen) // HD, :]
                            nc.gpsimd.tensor_tensor(
                                dgt, src, rstd_s, op=mybir.AluOpType.mult
                            )
                else:
                    # v chunk: bias add, cast bf16, scatter into 65-strided vext
                    hs = (c0 - 2 * D) // HD
                    nh = cs // HD
                    nc.vector.tensor_tensor(
                        vext[:, tt, hs:hs + nh, 0:HD],
                        psum.rearrange("p (s h) -> p s h", h=HD),
                        qkvb_bc[:, c0:c0 + cs].rearrange("p (s h) -> p s h", h=HD),
                        op=mybir.AluOpType.add,
                    )

        # ---- per-head attention (head pairs share one [128,.] transpose) ----
        attnout = data.tile([P, NT, D], BF16)  # [t_in, t_out, o]
        for hp in range(H // 2):
            # qqT/kkT: [hd, t] per head pair; head 2hp in partitions 0:64,
            # head 2hp+1 in partitions 64:128
            qqT = qkpool.tile([P, N], BF16, tag="qqT")
            kkT = qkpool.tile([P, N], BF16, tag="kkT")
            for (g0, gn) in _groups_of(NT, 4):
                srcs_q = [qn[:, g0 + i, hp * P:(hp + 1) * P] for i in range(gn)]
                pe_transpose_batch(srcs_q, qqT, g0 * P, nc.vector)
                srcs_k = [kn[:, g0 + i, hp * P:(hp + 1) * P] for i in range(gn)]
                pe_transpose_batch(srcs_k, kkT, g0 * P, nc.vector)
            for hh in range(2):
                h = hp * 2 + hh
                qT = qqT[hh * HD:(hh + 1) * HD, :]
                kT = kkT[hh * HD:(hh + 1) * HD, :]
                # E[j, i] = exp(scale * k_j . q_i)
                E = epool.tile([P, NT, N], BF16, tag="E")
                for jt in range(NT):
                    for ic in range(2):
                        ps = ps_st.tile([P, 512], F32, tag="st")
                        nc.tensor.matmul(
                            ps,
                            lhsT=kT[:, jt * P:(jt + 1) * P],
                            rhs=qT[:, ic * 512:(ic + 1) * 512],
                            start=True,
                            stop=True,
                        )
                        nc.scalar.activation(
                            E[:, jt, ic * 512:(ic + 1) * 512],
                            ps,
                            mybir.ActivationFunctionType.Exp,
                            scale=SCALE,
                        )
                # out[i, :] = (sum_j E[j,i] * [v_j | 1]); normalize by col 64
                for it in range(NT):
                    pa = ps_av.tile([P, HD + 1], F32, tag="av")
                    for jt in range(NT):
                        nc.tensor.matmul(
                            pa,
                            lhsT=E[:, jt, it * P:(it + 1) * P],
                            rhs=vext[:, jt, h, :],
                            start=(jt == 0),
                            stop=(jt == NT - 1),
                        )
                    rcp = stat.tile([P, 1], F32, tag="rcp")
                    nc.vector.reciprocal(rcp, pa[:, HD:HD + 1])
                    nc.vector.tensor_scalar_mul(
                        attnout[:, it, h * HD:(h + 1) * HD], pa[:, 0:HD], scalar1=rcp
                    )

        # ---- output projection ----
        attnoutT = data.tile([P, DC, N], BF16)  # [o_in, o_out, t]
        for oc in range(DC):
            for (g0, gn) in _groups_of(NT, 4):
                srcs = [attnout[:, g0 + i, oc * P:(oc + 1) * P] for i in range(gn)]
                pe_transpose_batch(srcs, attnoutT[:, oc, :], g0 * P, nc.vector)
        EC = 384
        for tt in range(NT):
            for ec in range(D // EC):
                ps_full = ps_mm.tile([P, 512], F32, tag="mm", name="ps_proj")
                ps = ps_full[:, :EC]
                for oc in range(DC):
                    nc.tensor.matmul(
                        ps,
                        lhsT=attnoutT[:, oc, tt * P:(tt + 1) * P],
                        rhs=projwT[:, oc, ec * EC:(ec + 1) * EC],
                        start=(oc == 0),
                        stop=(oc == DC - 1),
                    )
                ot = outp.tile([P, EC], F32, tag="outt")
                nc.vector.tensor_add(ot[:], ps, projb_bc[:, ec * EC:(ec + 1) * EC])
                nc.sync.dma_start(out_d[tt * P:(tt + 1) * P, ec * EC:(ec + 1) * EC], ot[:])


_NC_CACHE = {}


def _get_nc(apply_gn=True):
    if apply_gn not in _NC_CACHE:
        _NC_CACHE[apply_gn] = _build_graph(apply_gn)
    return _NC_CACHE[apply_gn]


def kernel(x, qkv_w, qkv_b, proj_w, proj_b, qn_gamma, qn_beta):
    qn_gamma = np.ascontiguousarray(qn_gamma, np.float32)
    qn_beta = np.ascontiguousarray(qn_beta, np.float32)
    apply_gn = not (np.all(qn_gamma == 1.0) and np.all(qn_beta == 0.0))
    nc = _get_nc(apply_gn)
    shared = {
        "qkv_w": np.ascontiguousarray(qkv_w, np.float32),
        "qkv_b": np.ascontiguousarray(qkv_b, np.float32),
        "proj_w": np.ascontiguousarray(proj_w, np.float32),
        "proj_b": np.ascontiguousarray(proj_b, np.float32),
        "qn_gamma": qn_gamma,
        "qn_beta": qn_beta,
    }
    in_maps = [
        {**shared, "x": np.ascontiguousarray(x[i], np.float32)} for i in range(B)
    ]
    res = run_bass_kernel_spmd(nc, in_maps, core_ids=list(range(B)))
    return np.stack([res.results[i]["out"] for i in range(B)], axis=0)
